# revision 10
# baseline (speedup 1.0000x reference)
"""RLeaky SNN scan kernel for Trainium2 (8 NeuronCores, batch data-parallel).

B=256, T=128, F=2048; per core: B_local=32 batch rows, full T=128 scan.

Layout: "f-major packed" [128, 512] tiles:
    tile[p, 32*fb + i] = logical element (batch row i, feature f = 128*fb + p)
(p = partition, fb = 0..15 feature block, i = 0..31 batch row) — the
transposed orientation the recurrent dot needs, so the matmul consumes
spk^T and produces dot^T with no per-step transposes.

The recurrent GEMM uses fp16 two-plane weights:
    W.T = hi + lo * 2^-11,  hi = fp16(W.T), lo = fp16((W.T - hi) * 2^11)
This representation is near-exact (~2^-24 relative residual; no fp16
subnormals since |W| ~ 2^-5.5) and every matmul product is exact: the
moving operands are spikes {0,1} (hi plane) and {0, 2^-11} (lo plane, a
power of two), so products are just exponent-shifted plane values,
accumulated in fp32 PSUM.  The only deviation from an fp32 evaluation is
the summation regrouping (hi-sum + lo-sum), measured at 4.3e-3 relative
error over the full chaotic scan (gate 2e-2) and 737 spike flips from
the fused-fp32 PE evaluation.

Why fp16 planes: fused fp32 matmuls self-load their stationary operand at
~427 ns per [128,128] block (no FastWeightLoad for fp32) — 109 us/step.
fp16 stationary gets FWL: ~35 ns per fused matmul, 17.9 us/step for both
planes (6.1x).  Measured end-to-end: 19.3 us/step, 2.47 ms per T=128 scan.

Per step (group g = tile columns 128g:128g+128 = fb blocks 4g..4g+3):
  hi_ps[g] += WTH(jb,fb).T @ spk16(jb)    jb = 0..15 ascending  (PE)
  lo_ps[g] += WTL(jb,fb).T @ spk16s(jb)   jb = 0..15 ascending  (PE)
  u3a = u2[g] + hi_ps[g]; u3 = u3a + lo_ps[g]; u4 = u3 + b[g]   (DVE)
  mem'[g] = u4 - spk[g];  spk'[g] = (mem'[g] > 1)               (DVE)
  spk16'[g] = fp16(spk'[g]); spk16s'[g] = fp16(spk'[g] * 2^-11) (DVE)
  u1' = 0.95*mem' (ACT); u2' = u1' + x_{t+1} (DVE)   [off critical path]
The 8 PSUM banks hold hi/lo x 4 groups; the DVE consumes group g while
the PE accumulates later groups, and step t+1's matmuls only wait for the
spk group their jb contracts with — the PE never idles.
"""

import sys

if "/opt/trn_rl_repo" not in sys.path:
    sys.path.insert(0, "/opt/trn_rl_repo")

import numpy as np

import concourse.mybir as mybir
import concourse.tile as tile
from concourse import bacc
from concourse.bass_utils import run_bass_kernel_spmd

F32 = mybir.dt.float32
F16 = mybir.dt.float16

B, T_FULL, F = 256, 128, 2048
NCORES = 8
BL = B // NCORES  # 32 batch rows per core
LOSC = float(2.0**-11)

_nc_cache = {}


def _emit_prologue(nc, pools, xp_d, wth_d, wtl_d, bp_d, x_index):
    """Emit W/b staging + state init; returns a mutable context dict."""
    wpool, wdma, const, state, xtp, u2p, tmp, pmm = pools

    wth_sb = wpool.tile([128, 16 * F], F16, name="wth_sb")
    wtl_sb = wpool.tile([128, 16 * F], F16, name="wtl_sb")
    for jb in range(16):
        for src, dst in ((wth_d, wth_sb), (wtl_d, wtl_sb)):
            wchunk = wdma.tile([128, F], F16, tag="wchunk")
            nc.gpsimd.dma_start(wchunk[:], src[jb * 128 : (jb + 1) * 128, :])
            nc.vector.tensor_copy(dst[:, jb * F : (jb + 1) * F], wchunk[:])

    bp_sb = const.tile([128, 512], F32)
    nc.gpsimd.dma_start(bp_sb[:], bp_d[:])

    mem_cur = state.tile([128, 512], F32, tag="mem", name="mem0")
    nc.vector.memset(mem_cur[:], 0.0)
    spk_cur = state.tile([128, 512], F32, tag="spk", name="spk0")
    nc.vector.memset(spk_cur[:], 0.0)
    s16_cur = state.tile([128, 512], F16, tag="s16", name="s16_0")
    nc.vector.memset(s16_cur[:], 0.0)
    s16s_cur = state.tile([128, 512], F16, tag="s16s", name="s16s_0")
    nc.vector.memset(s16s_cur[:], 0.0)

    x0 = xtp.tile([128, 512], F32, tag="xt", name="xt0")
    nc.gpsimd.dma_start(x0[:], xp_d[x_index(0), :, :])
    u1_t = tmp.tile([128, 512], F32, tag="u1", name="u1_0")
    nc.scalar.mul(u1_t[:], mem_cur[:], 0.95)
    u2_cur = u2p.tile([128, 512], F32, tag="u2", name="u2_0")
    nc.vector.tensor_add(u2_cur[:], u1_t[:], x0[:])

    return {
        "wth_sb": wth_sb, "wtl_sb": wtl_sb, "bp_sb": bp_sb,
        "mem": mem_cur, "spk": spk_cur, "s16": s16_cur, "s16s": s16s_cur,
        "u2": u2_cur,
    }


def _emit_steps(nc, pools, ctx, xp_d, steps, rec_spk, rec_mem, x_index,
                T_guard=None, k0=0):
    """Emit `steps` scan steps, mutating ctx's state tiles."""
    wpool, wdma, const, state, xtp, u2p, tmp, pmm = pools
    wth_sb, wtl_sb, bp_sb = ctx["wth_sb"], ctx["wtl_sb"], ctx["bp_sb"]
    mem_cur, spk_cur = ctx["mem"], ctx["spk"]
    s16_cur, s16s_cur, u2_cur = ctx["s16"], ctx["s16s"], ctx["u2"]

    def step(k, prefetch_next):
        nonlocal mem_cur, spk_cur, s16_cur, s16s_cur, u2_cur
        mem_new = state.tile([128, 512], F32, tag="mem", name=f"mem{k + 1}")
        spk_new = state.tile([128, 512], F32, tag="spk", name=f"spk{k + 1}")
        s16_new = state.tile([128, 512], F16, tag="s16", name=f"s16_{k + 1}")
        s16s_new = state.tile([128, 512], F16, tag="s16s", name=f"s16s_{k + 1}")

        for g in range(4):
            hi_ps = pmm.tile([128, 128], F32, tag=f"hi{g}", name=f"hi{k}_{g}")
            lo_ps = pmm.tile([128, 128], F32, tag=f"lo{g}", name=f"lo{k}_{g}")
            for fbi in range(4):
                fb = 4 * g + fbi
                for jb in range(16):
                    nc.tensor.matmul(
                        hi_ps[:, 32 * fbi : 32 * (fbi + 1)],
                        wth_sb[:, jb * F + fb * 128 : jb * F + fb * 128 + 128],
                        s16_cur[:, 32 * jb : 32 * (jb + 1)],
                        start=(jb == 0),
                        stop=(jb == 15),
                    )
            for fbi in range(4):
                fb = 4 * g + fbi
                for jb in range(16):
                    nc.tensor.matmul(
                        lo_ps[:, 32 * fbi : 32 * (fbi + 1)],
                        wtl_sb[:, jb * F + fb * 128 : jb * F + fb * 128 + 128],
                        s16s_cur[:, 32 * jb : 32 * (jb + 1)],
                        start=(jb == 0),
                        stop=(jb == 15),
                    )

            sl = slice(128 * g, 128 * (g + 1))
            u3a = tmp.tile([128, 128], F32, tag=f"u3a_{g}", name=f"u3a_{k}_{g}")
            nc.vector.tensor_add(u3a[:], u2_cur[:, sl], hi_ps[:])
            u3 = tmp.tile([128, 128], F32, tag=f"u3_{g}", name=f"u3_{k}_{g}")
            nc.vector.tensor_add(u3[:], u3a[:], lo_ps[:])
            u4 = tmp.tile([128, 128], F32, tag=f"u4_{g}", name=f"u4_{k}_{g}")
            nc.vector.tensor_add(u4[:], u3[:], bp_sb[:, sl])
            nc.vector.tensor_sub(mem_new[:, sl], u4[:], spk_cur[:, sl])
            nc.vector.tensor_scalar(
                spk_new[:, sl], mem_new[:, sl], 1.0, None, mybir.AluOpType.is_gt
            )
            nc.vector.tensor_copy(s16_new[:, sl], spk_new[:, sl])
            nc.vector.tensor_scalar(
                s16s_new[:, sl], spk_new[:, sl], LOSC, None, mybir.AluOpType.mult
            )

        nc.gpsimd.dma_start(rec_mem[k % rec_mem.shape[0], :, :], mem_new[:])
        nc.gpsimd.dma_start(rec_spk[k % rec_spk.shape[0], :, :], spk_new[:])

        if prefetch_next:
            xt1 = xtp.tile([128, 512], F32, tag="xt", name=f"xt{k + 1}")
            nc.gpsimd.dma_start(xt1[:], xp_d[x_index(k + 1), :, :])
            u1n = tmp.tile([128, 512], F32, tag="u1", name=f"u1_{k + 1}")
            nc.scalar.mul(u1n[:], mem_new[:], 0.95)
            u2n = u2p.tile([128, 512], F32, tag="u2", name=f"u2_{k + 1}")
            nc.vector.tensor_add(u2n[:], u1n[:], xt1[:])
            u2_cur = u2n

        mem_cur, spk_cur, s16_cur, s16s_cur = mem_new, spk_new, s16_new, s16s_new

    for k in range(steps):
        step(k, prefetch_next=(T_guard is None or k + 1 < T_guard))
    return lambda: mem_cur


def _pools(tc):
    return (
        tc.tile_pool(name="wpool", bufs=1),
        tc.tile_pool(name="wdma", bufs=2),
        tc.tile_pool(name="const", bufs=1),
        tc.tile_pool(name="state", bufs=2),
        tc.tile_pool(name="xtp", bufs=3),
        tc.tile_pool(name="u2p", bufs=2),
        tc.tile_pool(name="tmp", bufs=2),
        tc.tile_pool(name="pmm", bufs=1, space="PSUM"),
    )


def _build(T=T_FULL):
    if T in _nc_cache:
        return _nc_cache[T]

    nc = bacc.Bacc(None, target_bir_lowering=False)
    xp_d = nc.dram_tensor("xp", [T, 128, 512], F32, kind="ExternalInput")
    wth_d = nc.dram_tensor("wth", [F, F], F16, kind="ExternalInput")
    wtl_d = nc.dram_tensor("wtl", [F, F], F16, kind="ExternalInput")
    bp_d = nc.dram_tensor("bp", [128, 512], F32, kind="ExternalInput")
    spk_out = nc.dram_tensor("spk_out", [T, 128, 512], F32, kind="ExternalOutput")
    mem_out = nc.dram_tensor("mem_out", [T, 128, 512], F32, kind="ExternalOutput")

    with tile.TileContext(nc) as tc:
        cms = _pools(tc)
        pools = [cm.__enter__() for cm in cms]
        ctx = _emit_prologue(nc, pools, xp_d, wth_d, wtl_d, bp_d, x_index=lambda k: k)
        _emit_steps(
            nc, pools, ctx, xp_d, T, spk_out, mem_out,
            x_index=lambda k: k, T_guard=T,
        )
        for cm in reversed(cms):
            cm.__exit__(None, None, None)

    nc.compile()
    _nc_cache[T] = nc
    return nc


def _pack_x(xc, T):
    # [32, T, 2048] -> [T, 128, 512]: out[t, p, 32*fb + i] = xc[i, t, 128*fb + p]
    a = xc.transpose(1, 2, 0)  # [T, 2048, 32]
    a = a.reshape(T, 16, 128, 32).transpose(0, 2, 1, 3)  # [T, 128, 16, 32]
    return np.ascontiguousarray(a.reshape(T, 128, 512))


def _unpack_into(dst, a, T):
    # [T, 128, 512] packed -> dst[i, t, f] (one strided pass)
    v = a.reshape(T, 128, 16, 32).transpose(0, 2, 1, 3).reshape(T, 2048, 32)
    np.copyto(dst, v.transpose(2, 0, 1))


def _split_planes(Wt):
    hi = Wt.astype(np.float16)
    lo = ((Wt - hi.astype(np.float32)) * np.float32(2.0**11)).astype(np.float16)
    return hi, lo


def kernel(x, W, b, T=None, trace=False):
    x = np.asarray(x, dtype=np.float32)
    W = np.asarray(W, dtype=np.float32)
    b = np.asarray(b, dtype=np.float32)
    if T is None:
        T = x.shape[1]
    x = x[:, :T, :]

    nc = _build(T)
    Wt = np.ascontiguousarray(W.T)
    hi, lo = _split_planes(Wt)
    bp = np.ascontiguousarray(
        np.repeat(b.reshape(16, 128).T[:, :, None], 32, axis=2).reshape(128, 512)
    )

    in_maps = []
    for c in range(NCORES):
        xc = x[c * BL : (c + 1) * BL]  # [32, T, 2048]
        in_maps.append({"xp": _pack_x(xc, T), "wth": hi, "wtl": lo, "bp": bp})

    try:
        res = run_bass_kernel_spmd(
            nc, in_maps, core_ids=list(range(NCORES)), trace=trace
        )
    except ModuleNotFoundError:
        # no axon NTFF profiling hook in this environment; run without trace
        res = run_bass_kernel_spmd(
            nc, in_maps, core_ids=list(range(NCORES)), trace=False
        )

    spk_rec = np.empty((B, T, F), np.float32)
    mem_rec = np.empty((B, T, F), np.float32)
    for c in range(NCORES):
        _unpack_into(spk_rec[c * BL : (c + 1) * BL], res.results[c]["spk_out"], T)
        _unpack_into(mem_rec[c * BL : (c + 1) * BL], res.results[c]["mem_out"], T)
    if trace:
        kernel.last_result = res
    return spk_rec, mem_rec


# ---------------------------------------------------------------------------
# Timing build: For_i hardware loop over an 8-step body with identical
# per-step work (x DMA in, spk/mem records out) — NEFF size is constant in
# the iteration count, so wall-clock slope between two iteration counts is
# pure hardware step time (host/PJRT dispatch and NEFF-load cancel).
# ---------------------------------------------------------------------------

def _build_timing(iters, nsub=8):
    nc = bacc.Bacc(None, target_bir_lowering=False)
    xp_d = nc.dram_tensor("xp", [nsub, 128, 512], F32, kind="ExternalInput")
    wth_d = nc.dram_tensor("wth", [F, F], F16, kind="ExternalInput")
    wtl_d = nc.dram_tensor("wtl", [F, F], F16, kind="ExternalInput")
    bp_d = nc.dram_tensor("bp", [128, 512], F32, kind="ExternalInput")
    rec_spk = nc.dram_tensor("rec_spk", [nsub, 128, 512], F32, kind="Internal")
    rec_mem = nc.dram_tensor("rec_mem", [nsub, 128, 512], F32, kind="Internal")
    st_out = nc.dram_tensor("st_out", [128, 512], F32, kind="ExternalOutput")

    with tile.TileContext(nc) as tc:
        cms = _pools(tc)
        pools = [cm.__enter__() for cm in cms]
        # W staging + state init once, before the loop; the For_i body holds
        # 8 scan steps whose tile-pool slots return to their starting
        # positions (all loop-carried pools have bufs dividing 8).
        ctx = _emit_prologue(nc, pools, xp_d, wth_d, wtl_d, bp_d,
                             x_index=lambda k: k % nsub)
        final = None
        with tc.For_i(0, iters):
            final = _emit_steps(
                nc, pools, ctx, xp_d, nsub, rec_spk, rec_mem,
                x_index=lambda k: k % nsub,
            )
        nc.gpsimd.dma_start(st_out[:], final()[:])
        for cm in reversed(cms):
            cm.__exit__(None, None, None)

    nc.compile()
    return nc


def timing_in_map(x, W, b):
    """Host-side inputs for a _build_timing NEFF (one core's shapes)."""
    Wt = np.ascontiguousarray(np.asarray(W, np.float32).T)
    hi, lo = _split_planes(Wt)
    bp = np.ascontiguousarray(
        np.repeat(np.asarray(b, np.float32).reshape(16, 128).T[:, :, None], 32, axis=2
                  ).reshape(128, 512)
    )
    xp = _pack_x(np.asarray(x, np.float32)[:BL, :8, :], 8)
    return {"xp": xp, "wth": hi, "wtl": lo, "bp": bp}
